# revision 4
# baseline (speedup 1.0000x reference)
"""Trainium2 Bass kernel for nn_Attention (cross-attention, B=2 S=2048 D=1024 H=16).

Sharding: 8 cores = data-parallel over batch (2) x tensor-parallel over head
groups (4 groups of 4 heads). Each core computes q/k/v projections for its
256 output dims plus softmax(QK^T)V for its 4 heads; outputs are disjoint
slices of the full output, gathered host-side (no collectives).

v3 structure:
  - All matmul operands bf16 (fp32 PSUM accumulation).
  - Score matmuls for a head PAIR run concurrently on PE row-groups (K=64:
    k_h0 rows 0-63, k_h1 rows 64-127; tile_position auto-derived).
  - One exp ACTIVATE per (pair, i-block, j-chunk) covers both heads.
  - Software-pipelined slot loop: scores(slot+1) is emitted BEFORE av(slot)
    so the in-order PE queue streams through the exp-semaphore wait; the
    ACT engine (the pacing engine at ~1.1us/slot) stays dense.
  - Projections: only q[o0,ib0]+k[o0,ib0] run up front (~10us to first exp);
    v and k-o0 chunks are emitted just-in-time inside the first i-block;
    all remaining chunks are pumped 2 matmuls/slot into PE slack.
  - Block tail: copy PSUM->SBUF (frees the accumulator), reshape the
    denominator row to 64 partitions via DMA, reciprocal there (iterative
    divide is 8 cyc/elem/lane -> 64 lanes make it ~0.2us), broadcast back
    through DRAM, multiply, store.
"""

import numpy as np
import ml_dtypes

import concourse.bass as bass
import concourse.mybir as mybir
import concourse.tile as tile
from concourse.bass_utils import run_bass_kernel_spmd

B, S, D, H = 2, 2048, 1024, 16
HD = D // H  # 64 head dim
N_CORES = 8
HG = 4  # head groups = cores per batch entry
DH = D // HG  # 256 output dims per core
HPC = H // HG  # 4 heads per core
NF = D // 128  # 8 feature (contraction) chunks
F32 = mybir.dt.float32
BF16 = mybir.dt.bfloat16
EXP = mybir.ActivationFunctionType.Exp
BF = ml_dtypes.bfloat16


def _split_excess_waits(nc, cap=1):
    """This container's walrus caps sync waits at 1/instruction. Hoist excess
    waits onto InstNoOps inserted just before the instruction (same engine)."""
    ctr = 0
    spread = [
        mybir.EngineType.SP,
        mybir.EngineType.Pool,
        mybir.EngineType.PE,
        mybir.EngineType.DVE,
        mybir.EngineType.Activation,
    ]
    for bb in nc.main_func.blocks:
        insts = list(bb.instructions)
        out = []
        changed = False
        for inst in insts:
            si = inst.sync_info
            waits = list(si.on_wait) if (si is not None and si.on_wait) else []
            if len(waits) > cap:
                changed = True
                is_tail = type(inst).__name__ == "InstDrain" and len(waits) > 6
                for i, w in enumerate(waits[:-cap]):
                    ctr += 1
                    eng = spread[i % len(spread)] if is_tail else inst.engine
                    out.append(
                        mybir.InstNoOp(
                            name=f"I-waitsplit-{ctr}",
                            sync_info=mybir.SyncInfo(on_wait=[w], on_update=[]),
                            engine=eng,
                            ins=[],
                            outs=[],
                        )
                    )
                inst.sync_info = mybir.SyncInfo(
                    on_wait=waits[-cap:], on_update=list(si.on_update or [])
                )
            out.append(inst)
        if changed:
            bb.instructions = out
    return ctr


def build_nc(s=S, split_waits=True):
    """One core's program (SPMD: all cores run it on their own shard)."""
    nj = s // 128  # j (key token) chunks
    PW = 512  # i-block width (one fp32 PSUM bank)
    nih = s // PW
    TOK = 512  # token chunk for streaming/projections
    ntt = s // TOK
    tpj = TOK // 128

    nc = bass.Bass()
    xT = nc.dram_tensor("xT", [D, s], BF16, kind="ExternalInput")
    cT = nc.dram_tensor("cT", [D, s], BF16, kind="ExternalInput")
    wall = nc.dram_tensor("wall", [3 * D, DH], BF16, kind="ExternalInput")
    onesd = nc.dram_tensor("onesd", [128, HPC], BF16, kind="ExternalInput")
    out = nc.dram_tensor("out", [DH, s], F32, kind="ExternalOutput")

    with tile.TileContext(nc) as tc:
        with (
            tc.tile_pool(name="w", bufs=1) as wpool,
            tc.tile_pool(name="stream", bufs=2 * ntt) as spool,
            tc.tile_pool(name="res", bufs=1) as rpool,
            tc.tile_pool(name="vabp", bufs=nj) as vpool,
            tc.tile_pool(name="et", bufs=6) as epool,
            tc.tile_pool(name="sm", bufs=2) as smpool,
            tc.tile_pool(name="pj", bufs=2, space="PSUM") as pj,
            tc.tile_pool(name="ps", bufs=2, space="PSUM") as ps,
            tc.tile_pool(name="pv", bufs=1, space="PSUM") as pvp,
            tc.tile_pool(name="dram", bufs=2, space="DRAM") as dpool,
        ):
            # resident weights [feat_part, tensor, feat_chunk, outdim]
            w_all = wpool.tile([128, 3, NF, DH], BF16, tag="wall")
            nc.sync.dma_start(
                w_all[:], wall.rearrange("(t f p) o -> p t f o", p=128, f=NF)
            )
            wq_sb, wk_sb, wv_sb = w_all[:, 0], w_all[:, 1], w_all[:, 2]
            ones_sb = wpool.tile([128, HPC], BF16, tag="ones")
            nc.sync.dma_start(ones_sb[:], onesd[:])

            xTr = xT.rearrange("(f p) t -> p f t", p=128)
            cTr = cT.rearrange("(f p) t -> p f t", p=128)

            # input streams: issue in need-order, split across the two HW DGE
            # queues (SP and ACT) so c (needed by k/v early) and x overlap.
            xt, ct = [None] * ntt, [None] * ntt

            def load(which, i, engine):
                src = xTr if which == "x" else cTr
                t = spool.tile([128, NF, TOK], BF16, tag="st", name=f"{which}t{i}")
                engine.dma_start(t[:], src[:, :, i * TOK : (i + 1) * TOK])
                (xt if which == "x" else ct)[i] = t

            load("x", 0, nc.scalar)  # q00 needs it first
            load("c", 0, nc.sync)
            load("c", 1, nc.scalar)
            load("c", 2, nc.sync)
            load("c", 3, nc.scalar)
            load("x", 1, nc.sync)
            load("x", 2, nc.scalar)
            load("x", 3, nc.sync)

            # q/k as per-(o, ib) tiles for fine-grained dependencies
            qts = [[None] * ntt for _ in range(2)]
            kts = [[None] * ntt for _ in range(2)]

            def proj_chunk(w_sb, toks, o, ib, dst, tag):
                t = rpool.tile(
                    [128, TOK], BF16, tag=f"{tag}{o}_{ib}", name=f"{tag}{o}_{ib}"
                )
                pq = pj.tile([128, TOK], F32, tag="pp", name="pq")
                for f in range(NF):
                    nc.tensor.matmul(
                        pq[:, :],
                        w_sb[:, f, o * 128 : (o + 1) * 128],
                        toks[ib][:, f, :],
                        start=(f == 0),
                        stop=(f == NF - 1),
                    )
                nc.vector.tensor_copy(t[:], pq[:, :])
                dst[o][ib] = t

            def chunk_thunks(w_sb, toks, o, ib, dst, tag):
                """Split one projection chunk into 8 per-matmul thunks for
                fine-grained interleaving into the attention stream."""
                holder = {}
                ths = []
                for f in range(NF):

                    def th(f=f):
                        if f == 0:
                            holder["t"] = rpool.tile(
                                [128, TOK],
                                BF16,
                                tag=f"{tag}{o}_{ib}",
                                name=f"{tag}{o}_{ib}",
                            )
                            holder["pq"] = pj.tile([128, TOK], F32, tag="pp", name="pq")
                        nc.tensor.matmul(
                            holder["pq"][:, :],
                            w_sb[:, f, o * 128 : (o + 1) * 128],
                            toks[ib][:, f, :],
                            start=(f == 0),
                            stop=(f == NF - 1),
                        )
                        if f == NF - 1:
                            nc.vector.tensor_copy(holder["t"][:], holder["pq"][:, :])
                            dst[o][ib] = holder["t"]

                    ths.append(th)
                return ths

            vab = [None] * nj

            def emit_v(jc):
                # v[j, o] = sum_f cT[f,j] * WvT[f,o]; + ones column -> denominator
                pvv = pj.tile([128, TOK], F32, tag="pp", name="pvv")
                for f in range(NF):
                    nc.tensor.matmul(
                        pvv[:, :DH],
                        ct[jc // tpj][:, f, (jc % tpj) * 128 : (jc % tpj + 1) * 128],
                        wv_sb[:, f, :],
                        start=(f == 0),
                        stop=(f == NF - 1),
                    )
                va = vpool.tile([128, HPC, HD + 1], BF16, tag="vab", name="va")
                nc.vector.tensor_copy(
                    va[:, :, :HD], pvv[:, :DH].rearrange("p (h c) -> p h c", c=HD)
                )
                nc.vector.tensor_copy(va[:, :, HD : HD + 1], ones_sb[:, :, None])
                vab[jc] = va

            # minimal upfront projections: first q and first k chunk only
            proj_chunk(wq_sb, xt, 0, 0, qts, "q")
            proj_chunk(wk_sb, ct, 0, 0, kts, "k")

            # remaining chunks pumped into PE slack during pair0 ih1..3
            pump_q = []
            for o, ib, wsb, tk, dst, tag in (
                (0, 2, wq_sb, xt, qts, "q"),  # needed end of ih1
                (0, 3, wq_sb, xt, qts, "q"),  # needed end of ih2
                (1, 0, wk_sb, ct, kts, "k"),  # pair1 ...
                (1, 0, wq_sb, xt, qts, "q"),
                (1, 1, wk_sb, ct, kts, "k"),
                (1, 1, wq_sb, xt, qts, "q"),
                (1, 2, wk_sb, ct, kts, "k"),
                (1, 2, wq_sb, xt, qts, "q"),
                (1, 3, wk_sb, ct, kts, "k"),
                (1, 3, wq_sb, xt, qts, "q"),
            ):
                pump_q.extend(chunk_thunks(wsb, tk, o, ib, dst, tag))
            pump_q = list(reversed(pump_q))  # pop() from the front via pop()

            def pump(n):
                for _ in range(n):
                    if pump_q:
                        pump_q.pop()()

            # ---- software-pipelined attention ----
            slots = [
                (pair, ih, jt)
                for pair in range(2)
                for ih in range(nih)
                for jt in range(nj)
            ]
            psc_map, et_map, ppv_map = {}, {}, {}

            def emit_scores(pair, ih, jt):
                psc = ps.tile([128, 2, PW], F32, tag="sc", name="psc")
                for hh in range(2):
                    pb = hh * 64
                    nc.tensor.matmul(
                        psc[:, hh, :],
                        kts[pair][jt // tpj][
                            pb : pb + 64, (jt % tpj) * 128 : (jt % tpj + 1) * 128
                        ],
                        qts[pair][ih][pb : pb + 64, :],
                        start=True,
                        stop=True,
                    )
                psc_map[(pair, ih, jt)] = psc

            def emit_exp(key):
                et = epool.tile([128, 2, PW], BF16, tag="et", name="et")
                nc.scalar.activation(et[:], psc_map.pop(key)[:], EXP)
                et_map[key] = et

            def emit_av(pair, ih, jt):
                ppv = ppv_map[(pair, ih)]
                et = et_map.pop((pair, ih, jt))
                for hh in range(2):
                    nc.tensor.matmul(
                        ppv[:, hh, :],
                        vab[jt][:, pair * 2 + hh, :],
                        et[:, hh, :],
                        start=(jt == 0),
                        stop=(jt == nj - 1),
                    )

            def emit_tail(pair, ih):
                ppv = ppv_map.pop((pair, ih))
                psb = smpool.tile([HD + 1, 2, PW], F32, tag="psb", name="psb")
                nc.vector.tensor_copy(psb[:], ppv[:])  # frees ppv PSUM
                # reshape denominator row [1, 1024] -> [64, 16] so the DVE's
                # iterative divide (8 cyc/elem/lane) runs on 64 lanes
                rdsq = smpool.tile([64, 16], F32, tag="rdsq", name="rdsq")
                nc.sync.dma_start(
                    rdsq[:], psb[HD : HD + 1, :, :].rearrange("p a b -> p (a b)")
                )
                rqi = smpool.tile([64, 16], F32, tag="rqi", name="rqi")
                nc.vector.reciprocal(rqi[:], rdsq[:])
                rdd = dpool.tile([1, 2 * PW], F32, tag="rdd", name="rdd")
                nc.sync.dma_start(rdd[:], rqi[:])
                rdb = smpool.tile([64, 2, PW], F32, tag="rdb", name="rdb")
                rsrc = rdd[0, :]
                bsrc = bass.AP(
                    tensor=rsrc.tensor,
                    offset=rsrc.offset,
                    ap=[[0, 64]] + list(rsrc.ap),
                )
                nc.sync.dma_start(rdb.rearrange("p a b -> p (a b)"), bsrc)
                ob = smpool.tile([64, 2, PW], F32, tag="ob", name="ob")
                nc.vector.tensor_mul(ob[:], psb[:HD, :, :], rdb[:])
                nc.sync.dma_start(
                    out[
                        pair * 128 : (pair + 1) * 128, ih * PW : (ih + 1) * PW
                    ].rearrange("(h c) i -> c h i", h=2),
                    ob[:],
                )

            emit_scores(*slots[0])
            for i, (pair, ih, jt) in enumerate(slots):
                first_block = pair == 0 and ih == 0
                if first_block:
                    emit_v(jt)  # needed by av this slot
                    if jt in (1, 5, 9):
                        proj_chunk(wk_sb, ct, 0, jt // 4 + 1, kts, "k")
                    if jt == 13:
                        proj_chunk(wq_sb, xt, 0, 1, qts, "q")
                if jt == 0:
                    ppv_map[(pair, ih)] = pvp.tile(
                        [HD + 1, 2, PW], F32, tag="pv", name="ppv"
                    )
                if i + 1 < len(slots):
                    emit_scores(*slots[i + 1])
                emit_exp((pair, ih, jt))
                if not first_block:
                    pump(2)
                emit_av(pair, ih, jt)
                if jt == nj - 1:
                    emit_tail(pair, ih)
            pump(len(pump_q))  # safety: flush anything left

    if split_waits:
        _split_excess_waits(nc)
    return nc


def make_in_maps(x, context, Wq, Wkv, s=S):
    """Host-side shard + layout prep. Core c -> (batch c//HG, head group c%HG)."""
    x = np.asarray(x, dtype=np.float32)
    context = np.asarray(context, dtype=np.float32)
    Wq = np.asarray(Wq, dtype=np.float32)
    Wkv = np.asarray(Wkv, dtype=np.float32)
    scale = np.float32(HD**-0.5)
    in_maps = []
    for core in range(N_CORES):
        b, hg = core // HG, core % HG
        sl = slice(hg * DH, (hg + 1) * DH)
        in_maps.append(
            {
                "xT": np.ascontiguousarray(x[b].T).astype(BF),
                "cT": np.ascontiguousarray(context[b].T).astype(BF),
                "wall": np.ascontiguousarray(
                    np.concatenate(
                        [
                            Wq[sl].T * scale,
                            Wkv[sl].T,
                            Wkv[D + hg * DH : D + (hg + 1) * DH].T,
                        ],
                        axis=0,
                    )
                ).astype(BF),
                "onesd": np.ones((128, HPC), dtype=BF),
            }
        )
    return in_maps


def gather_out(results, s=S):
    full = np.empty((B, s, D), dtype=np.float32)
    for core in range(N_CORES):
        b, hg = core // HG, core % HG
        full[b, :, hg * DH : (hg + 1) * DH] = results[core]["out"].T
    return full


def kernel(x, context, Wq, Wkv):
    nc = build_nc(S)
    in_maps = make_in_maps(x, context, Wq, Wkv, S)
    res = run_bass_kernel_spmd(nc, in_maps, list(range(N_CORES)))
    return gather_out(res.results, S)


# revision 5
# speedup vs baseline: 1.1898x; 1.1898x over previous
"""Trainium2 Bass kernel for nn_Attention (cross-attention, B=2 S=2048 D=1024 H=16).

Sharding: 8 cores = data-parallel over batch (2) x tensor-parallel over head
groups (4 groups of 4 heads). Each core computes q/k/v projections for its
256 output dims plus softmax(QK^T)V for its 4 heads; outputs are disjoint
slices of the full output, gathered host-side (no collectives).

v4 structure:
  - All matmul operands bf16 (fp32 PSUM accumulation).
  - Host pre-arranges x/context/weights so every DMA is contiguous per
    partition (8-16KB descriptor runs); weights split in three so Wq lands
    first and the first projection starts ~10us in.
  - PE warmup matmuls + a dummy activation (ACT table load) run during the
    input DMA window, so real work starts at full clock.
  - Score matmuls for a head PAIR run concurrently on PE row-groups (K=64:
    k_h0 rows 0-63, k_h1 rows 64-127; tile_position auto-derived).
  - One exp ACTIVATE per (pair, i-block, j-chunk) covers both heads; the
    ACT engine is the pacing engine (~1.1us/slot).
  - v and k chunks are emitted just-in-time inside the first i-block;
    remaining projection chunks are pumped 2 matmuls/slot into PE slack.
  - Block tail: copy PSUM->SBUF (frees the accumulator), reshape the
    denominator row to 64 partitions via DMA, reciprocal there (8 cyc/elem
    /lane on 64 lanes ~0.3us), broadcast back through DRAM, multiply, store.
"""

import numpy as np
import ml_dtypes

import concourse.bass as bass
import concourse.mybir as mybir
import concourse.tile as tile
from concourse.bass_utils import run_bass_kernel_spmd

B, S, D, H = 2, 2048, 1024, 16
HD = D // H  # 64 head dim
N_CORES = 8
HG = 4  # head groups = cores per batch entry
DH = D // HG  # 256 output dims per core
HPC = H // HG  # 4 heads per core
NF = D // 128  # 8 feature (contraction) chunks
TOK = 512  # token chunk for streaming/projections
NTT = S // TOK
F32 = mybir.dt.float32
BF16 = mybir.dt.bfloat16
EXP = mybir.ActivationFunctionType.Exp
BF = ml_dtypes.bfloat16


def _split_excess_waits(nc, cap=1):
    """This container's walrus caps sync waits at 1/instruction. Hoist excess
    waits onto InstNoOps inserted just before the instruction (same engine)."""
    ctr = 0
    spread = [
        mybir.EngineType.SP,
        mybir.EngineType.Pool,
        mybir.EngineType.PE,
        mybir.EngineType.DVE,
        mybir.EngineType.Activation,
    ]
    for bb in nc.main_func.blocks:
        insts = list(bb.instructions)
        out = []
        changed = False
        for inst in insts:
            si = inst.sync_info
            waits = list(si.on_wait) if (si is not None and si.on_wait) else []
            if len(waits) > cap:
                changed = True
                is_tail = type(inst).__name__ == "InstDrain" and len(waits) > 6
                for i, w in enumerate(waits[:-cap]):
                    ctr += 1
                    eng = spread[i % len(spread)] if is_tail else inst.engine
                    out.append(
                        mybir.InstNoOp(
                            name=f"I-waitsplit-{ctr}",
                            sync_info=mybir.SyncInfo(on_wait=[w], on_update=[]),
                            engine=eng,
                            ins=[],
                            outs=[],
                        )
                    )
                inst.sync_info = mybir.SyncInfo(
                    on_wait=waits[-cap:], on_update=list(si.on_update or [])
                )
            out.append(inst)
        if changed:
            bb.instructions = out
    return ctr


def build_nc(s=S, split_waits=True):
    """One core's program (SPMD: all cores run it on their own shard)."""
    nj = s // 128  # j (key token) chunks
    PW = 512  # i-block width (one fp32 PSUM bank)
    nih = s // PW
    ntt = s // TOK
    tpj = TOK // 128

    nc = bass.Bass()
    # host pre-arranged: every tensor contiguous per partition
    xT = nc.dram_tensor("xT", [128, ntt, NF, TOK], BF16, kind="ExternalInput")
    cT = nc.dram_tensor("cT", [128, ntt, NF, TOK], BF16, kind="ExternalInput")
    wqd = nc.dram_tensor("wqd", [128, NF, DH], BF16, kind="ExternalInput")
    wkd = nc.dram_tensor("wkd", [128, NF, DH], BF16, kind="ExternalInput")
    wvd = nc.dram_tensor("wvd", [128, NF, DH], BF16, kind="ExternalInput")
    onesd = nc.dram_tensor("onesd", [128, HPC], BF16, kind="ExternalInput")
    out = nc.dram_tensor("out", [DH, s], F32, kind="ExternalOutput")

    with tile.TileContext(nc) as tc:
        with (
            tc.tile_pool(name="w", bufs=1) as wpool,
            tc.tile_pool(name="stream", bufs=2 * ntt) as spool,
            tc.tile_pool(name="res", bufs=1) as rpool,
            tc.tile_pool(name="vabp", bufs=nj) as vpool,
            tc.tile_pool(name="et", bufs=6) as epool,
            tc.tile_pool(name="sm", bufs=2) as smpool,
            tc.tile_pool(name="pj", bufs=2, space="PSUM") as pj,
            tc.tile_pool(name="ps", bufs=2, space="PSUM") as ps,
            tc.tile_pool(name="pv", bufs=1, space="PSUM") as pvp,
            tc.tile_pool(name="dram", bufs=2, space="DRAM") as dpool,
        ):
            # ---- warmup: ramp the PE p-state + load the exp ACT table while
            # the input DMAs stream in ----
            junk = wpool.tile([128, 640], BF16, tag="junk")
            nc.gpsimd.memset(junk[:], 1.0)
            jact = wpool.tile([128, 8], F32, tag="jact")
            nc.scalar.activation(jact[:], junk[:, :8], EXP)
            for wi in range(12):
                wps = pj.tile([128, TOK], F32, tag="pp", name="wps")
                nc.tensor.matmul(
                    wps[:, :], junk[:, :128], junk[:, 128:640], start=True, stop=True
                )

            wq_sb = wpool.tile([128, NF, DH], BF16, tag="wq")
            nc.sync.dma_start(wq_sb[:], wqd[:])
            wk_sb = wpool.tile([128, NF, DH], BF16, tag="wk")
            nc.sync.dma_start(wk_sb[:], wkd[:])
            ones_sb = wpool.tile([128, HPC], BF16, tag="ones")
            nc.sync.dma_start(ones_sb[:], onesd[:])

            # input streams in need-order, split across the two HW DGE queues
            xt, ct = [None] * ntt, [None] * ntt
            wv_holder = {}

            def load(which, i, engine):
                if which == "wv":
                    t = wpool.tile([128, NF, DH], BF16, tag="wv")
                    engine.dma_start(t[:], wvd[:])
                    wv_holder["wv"] = t
                    return
                src = xT if which == "x" else cT
                t = spool.tile([128, NF, TOK], BF16, tag="st", name=f"{which}t{i}")
                engine.dma_start(t[:], src[:, i])
                (xt if which == "x" else ct)[i] = t

            load("x", 0, nc.scalar)
            load("wv", 0, nc.scalar)
            load("c", 0, nc.sync)
            load("c", 1, nc.scalar)
            load("c", 2, nc.sync)
            load("c", 3, nc.scalar)
            load("x", 1, nc.sync)
            load("x", 2, nc.scalar)
            load("x", 3, nc.sync)

            # q/k as per-(o, ib) tiles for fine-grained dependencies
            qts = [[None] * ntt for _ in range(2)]
            kts = [[None] * ntt for _ in range(2)]

            def proj_chunk(w_sb, toks, o, ib, dst, tag):
                t = rpool.tile(
                    [128, TOK], BF16, tag=f"{tag}{o}_{ib}", name=f"{tag}{o}_{ib}"
                )
                pq = pj.tile([128, TOK], F32, tag="pp", name="pq")
                for f in range(NF):
                    nc.tensor.matmul(
                        pq[:, :],
                        w_sb[:, f, o * 128 : (o + 1) * 128],
                        toks[ib][:, f, :],
                        start=(f == 0),
                        stop=(f == NF - 1),
                    )
                nc.vector.tensor_copy(t[:], pq[:, :])
                dst[o][ib] = t

            def chunk_thunks(w_sb, toks, o, ib, dst, tag):
                """One projection chunk as 8 per-matmul thunks for fine-grained
                interleaving into the attention stream."""
                holder = {}
                ths = []
                for f in range(NF):

                    def th(f=f):
                        if f == 0:
                            holder["t"] = rpool.tile(
                                [128, TOK],
                                BF16,
                                tag=f"{tag}{o}_{ib}",
                                name=f"{tag}{o}_{ib}",
                            )
                            holder["pq"] = pj.tile([128, TOK], F32, tag="pp", name="pq")
                        nc.tensor.matmul(
                            holder["pq"][:, :],
                            w_sb[:, f, o * 128 : (o + 1) * 128],
                            toks[ib][:, f, :],
                            start=(f == 0),
                            stop=(f == NF - 1),
                        )
                        if f == NF - 1:
                            nc.vector.tensor_copy(holder["t"][:], holder["pq"][:, :])
                            dst[o][ib] = holder["t"]

                    ths.append(th)
                return ths

            vab = [None] * nj

            def emit_v(jc):
                # v[j, o] = sum_f cT[f,j] * WvT[f,o]; + ones column -> denominator
                pvv = pj.tile([128, TOK], F32, tag="pp", name="pvv")
                for f in range(NF):
                    nc.tensor.matmul(
                        pvv[:, :DH],
                        ct[jc // tpj][:, f, (jc % tpj) * 128 : (jc % tpj + 1) * 128],
                        wv_holder["wv"][:, f, :],
                        start=(f == 0),
                        stop=(f == NF - 1),
                    )
                va = vpool.tile([128, HPC, HD + 1], BF16, tag="vab", name="va")
                nc.vector.tensor_copy(
                    va[:, :, :HD], pvv[:, :DH].rearrange("p (h c) -> p h c", c=HD)
                )
                nc.vector.tensor_copy(va[:, :, HD : HD + 1], ones_sb[:, :, None])
                vab[jc] = va

            # minimal upfront projections: first q and first k chunk only
            proj_chunk(wq_sb, xt, 0, 0, qts, "q")
            proj_chunk(wk_sb, ct, 0, 0, kts, "k")

            # remaining chunks pumped into PE slack during pair0 ih1..3
            pump_q = []
            for o, ib, wsb, tk, dst, tag in (
                (0, 2, wq_sb, xt, qts, "q"),  # needed end of ih1
                (0, 3, wq_sb, xt, qts, "q"),  # needed end of ih2
                (1, 0, wk_sb, ct, kts, "k"),  # pair1 ...
                (1, 0, wq_sb, xt, qts, "q"),
                (1, 1, wk_sb, ct, kts, "k"),
                (1, 1, wq_sb, xt, qts, "q"),
                (1, 2, wk_sb, ct, kts, "k"),
                (1, 2, wq_sb, xt, qts, "q"),
                (1, 3, wk_sb, ct, kts, "k"),
                (1, 3, wq_sb, xt, qts, "q"),
            ):
                pump_q.extend(chunk_thunks(wsb, tk, o, ib, dst, tag))
            pump_q = list(reversed(pump_q))

            def pump(n):
                for _ in range(n):
                    if pump_q:
                        pump_q.pop()()

            # ---- attention: slot per (pair, i-block, j-chunk) ----
            slots = [
                (pair, ih, jt)
                for pair in range(2)
                for ih in range(nih)
                for jt in range(nj)
            ]
            ppv_map = {}

            def emit_tail(pair, ih):
                ppv = ppv_map.pop((pair, ih))
                psb = smpool.tile([HD + 1, 2, PW], F32, tag="psb", name="psb")
                nc.vector.tensor_copy(psb[:], ppv[:])  # frees ppv PSUM
                # reshape denominator row [1, 1024] -> [64, 16] so the DVE's
                # iterative divide (8 cyc/elem/lane) runs on 64 lanes
                rdsq = smpool.tile([64, 16], F32, tag="rdsq", name="rdsq")
                nc.gpsimd.dma_start(
                    rdsq[:], psb[HD : HD + 1, :, :].rearrange("p a b -> p (a b)")
                )
                rqi = smpool.tile([64, 16], F32, tag="rqi", name="rqi")
                nc.vector.reciprocal(rqi[:], rdsq[:])
                rdd = dpool.tile([1, 2 * PW], F32, tag="rdd", name="rdd")
                nc.gpsimd.dma_start(rdd[:], rqi[:])
                rdb = smpool.tile([64, 2, PW], F32, tag="rdb", name="rdb")
                rsrc = rdd[0, :]
                bsrc = bass.AP(
                    tensor=rsrc.tensor,
                    offset=rsrc.offset,
                    ap=[[0, 64]] + list(rsrc.ap),
                )
                nc.gpsimd.dma_start(rdb.rearrange("p a b -> p (a b)"), bsrc)
                ob = smpool.tile([64, 2, PW], F32, tag="ob", name="ob")
                nc.vector.tensor_mul(ob[:], psb[:HD, :, :], rdb[:])
                nc.sync.dma_start(
                    out[
                        pair * 128 : (pair + 1) * 128, ih * PW : (ih + 1) * PW
                    ].rearrange("(h c) i -> c h i", h=2),
                    ob[:],
                )

            for pair, ih, jt in slots:
                first_block = pair == 0 and ih == 0
                if first_block:
                    emit_v(jt)  # needed by av this slot
                    if jt in (1, 5, 9):
                        proj_chunk(wk_sb, ct, 0, jt // 4 + 1, kts, "k")
                    if jt == 13:
                        proj_chunk(wq_sb, xt, 0, 1, qts, "q")
                if jt == 0:
                    ppv_map[(pair, ih)] = pvp.tile(
                        [HD + 1, 2, PW], F32, tag="pv", name="ppv"
                    )
                psc = ps.tile([128, 2, PW], F32, tag="sc", name="psc")
                for hh in range(2):
                    pb = hh * 64
                    nc.tensor.matmul(
                        psc[:, hh, :],
                        kts[pair][jt // tpj][
                            pb : pb + 64, (jt % tpj) * 128 : (jt % tpj + 1) * 128
                        ],
                        qts[pair][ih][pb : pb + 64, :],
                        start=True,
                        stop=True,
                    )
                et = epool.tile([128, 2, PW], BF16, tag="et", name="et")
                nc.scalar.activation(et[:], psc[:], EXP)
                if not first_block:
                    pump(2)
                ppv = ppv_map[(pair, ih)]
                for hh in range(2):
                    nc.tensor.matmul(
                        ppv[:, hh, :],
                        vab[jt][:, pair * 2 + hh, :],
                        et[:, hh, :],
                        start=(jt == 0),
                        stop=(jt == nj - 1),
                    )
                if jt == nj - 1:
                    emit_tail(pair, ih)
            pump(len(pump_q))  # safety: flush anything left

    if split_waits:
        _split_excess_waits(nc)
    return nc


def make_in_maps(x, context, Wq, Wkv, s=S):
    """Host-side shard + layout prep. Core c -> (batch c//HG, head group c%HG).

    x/context ship as [128, ntt, NF, TOK] and weights as [128, NF, DH] so all
    DMA reads are contiguous per partition."""
    x = np.asarray(x, dtype=np.float32)
    context = np.asarray(context, dtype=np.float32)
    Wq = np.asarray(Wq, dtype=np.float32)
    Wkv = np.asarray(Wkv, dtype=np.float32)
    scale = np.float32(HD**-0.5)
    ntt = s // TOK

    def arrange_tok(a):  # [s, D] -> [128, ntt, NF, TOK]
        return np.ascontiguousarray(
            a.T.reshape(NF, 128, ntt, TOK).transpose(1, 2, 0, 3)
        ).astype(BF)

    def arrange_w(w):  # [DH, D] -> [128, NF, DH]
        return np.ascontiguousarray(w.T.reshape(NF, 128, DH).transpose(1, 0, 2)).astype(
            BF
        )

    in_maps = []
    for core in range(N_CORES):
        b, hg = core // HG, core % HG
        sl = slice(hg * DH, (hg + 1) * DH)
        in_maps.append(
            {
                "xT": arrange_tok(x[b]),
                "cT": arrange_tok(context[b]),
                "wqd": arrange_w(Wq[sl] * scale),
                "wkd": arrange_w(Wkv[sl]),
                "wvd": arrange_w(Wkv[D + hg * DH : D + (hg + 1) * DH]),
                "onesd": np.ones((128, HPC), dtype=BF),
            }
        )
    return in_maps


def gather_out(results, s=S):
    full = np.empty((B, s, D), dtype=np.float32)
    for core in range(N_CORES):
        b, hg = core // HG, core % HG
        full[b, :, hg * DH : (hg + 1) * DH] = results[core]["out"].T
    return full


def kernel(x, context, Wq, Wkv):
    nc = build_nc(S)
    in_maps = make_in_maps(x, context, Wq, Wkv, S)
    res = run_bass_kernel_spmd(nc, in_maps, list(range(N_CORES)))
    return gather_out(res.results, S)


# revision 13
# speedup vs baseline: 1.2064x; 1.0139x over previous
"""Trainium2 Bass kernel for nn_Attention (cross-attention, B=2 S=2048 D=1024 H=16).

Sharding: 8 cores = data-parallel over batch (2) x tensor-parallel over head
groups (4 groups of 4 heads). Each core computes q/k/v projections for its
256 output dims plus softmax(QK^T)V for its 4 heads; outputs are disjoint
slices of the full output, gathered host-side (no collectives).

v4 structure:
  - All matmul operands bf16 (fp32 PSUM accumulation).
  - Host pre-arranges x/context/weights so every DMA is contiguous per
    partition (8-16KB descriptor runs); weights split in three so Wq lands
    first and the first projection starts ~10us in.
  - PE warmup matmuls + a dummy activation (ACT table load) run during the
    input DMA window, so real work starts at full clock.
  - Score matmuls for a head PAIR run concurrently on PE row-groups (K=64:
    k_h0 rows 0-63, k_h1 rows 64-127; tile_position auto-derived).
  - One exp ACTIVATE per (pair, i-block, j-chunk) covers both heads; the
    ACT engine is the pacing engine (~1.1us/slot).
  - v and k chunks are emitted just-in-time inside the first i-block;
    remaining projection chunks are pumped 2 matmuls/slot into PE slack.
  - Block tail: copy PSUM->SBUF (frees the accumulator), reshape the
    denominator row to 64 partitions via DMA, reciprocal there (8 cyc/elem
    /lane on 64 lanes ~0.3us), broadcast back through DRAM, multiply, store.
"""

import numpy as np
import ml_dtypes

import concourse.bass as bass
import concourse.mybir as mybir
import concourse.tile as tile
from concourse.bass_utils import run_bass_kernel_spmd

B, S, D, H = 2, 2048, 1024, 16
HD = D // H  # 64 head dim
N_CORES = 8
HG = 4  # head groups = cores per batch entry
DH = D // HG  # 256 output dims per core
HPC = H // HG  # 4 heads per core
NF = D // 128  # 8 feature (contraction) chunks
TOK = 512  # token chunk for streaming/projections
NTT = S // TOK
F32 = mybir.dt.float32
BF16 = mybir.dt.bfloat16
EXP = mybir.ActivationFunctionType.Exp
BF = ml_dtypes.bfloat16


def _split_excess_waits(nc, cap=1):
    """This container's walrus caps sync waits at 1/instruction. Hoist excess
    waits onto InstNoOps inserted just before the instruction (same engine)."""
    ctr = 0
    spread = [
        mybir.EngineType.SP,
        mybir.EngineType.Pool,
        mybir.EngineType.PE,
        mybir.EngineType.DVE,
        mybir.EngineType.Activation,
    ]
    for bb in nc.main_func.blocks:
        insts = list(bb.instructions)
        out = []
        changed = False
        for inst in insts:
            si = inst.sync_info
            waits = list(si.on_wait) if (si is not None and si.on_wait) else []
            if len(waits) > cap:
                changed = True
                is_tail = type(inst).__name__ == "InstDrain" and len(waits) > 6
                for i, w in enumerate(waits[:-cap]):
                    ctr += 1
                    eng = spread[i % len(spread)] if is_tail else inst.engine
                    out.append(
                        mybir.InstNoOp(
                            name=f"I-waitsplit-{ctr}",
                            sync_info=mybir.SyncInfo(on_wait=[w], on_update=[]),
                            engine=eng,
                            ins=[],
                            outs=[],
                        )
                    )
                inst.sync_info = mybir.SyncInfo(
                    on_wait=waits[-cap:], on_update=list(si.on_update or [])
                )
            out.append(inst)
        if changed:
            bb.instructions = out
    return ctr


def build_nc(s=S, split_waits=True):
    """One core's program (SPMD: all cores run it on their own shard)."""
    nj = s // 128  # j (key token) chunks
    PW = 512  # i-block width (one fp32 PSUM bank)
    nih = s // PW
    ntt = s // TOK
    tpj = TOK // 128

    nc = bass.Bass()
    # host pre-arranged: every tensor contiguous per partition
    xT = nc.dram_tensor("xT", [128, ntt, NF, TOK], BF16, kind="ExternalInput")
    cT = nc.dram_tensor("cT", [128, ntt, NF, TOK], BF16, kind="ExternalInput")
    wqd = nc.dram_tensor("wqd", [128, NF, DH], BF16, kind="ExternalInput")
    wkd = nc.dram_tensor("wkd", [128, NF, DH], BF16, kind="ExternalInput")
    wvd = nc.dram_tensor("wvd", [128, NF, DH], BF16, kind="ExternalInput")
    onesd = nc.dram_tensor("onesd", [128, HPC], BF16, kind="ExternalInput")
    out = nc.dram_tensor("out", [DH, s], F32, kind="ExternalOutput")

    with tile.TileContext(nc) as tc:
        with (
            tc.tile_pool(name="w", bufs=1) as wpool,
            tc.tile_pool(name="stream", bufs=2 * ntt) as spool,
            tc.tile_pool(name="res", bufs=1) as rpool,
            tc.tile_pool(name="vabp", bufs=nj) as vpool,
            tc.tile_pool(name="et", bufs=6) as epool,
            tc.tile_pool(name="sm", bufs=2) as smpool,
            tc.tile_pool(name="pj", bufs=2, space="PSUM") as pj,
            tc.tile_pool(name="ps", bufs=2, space="PSUM") as ps,
            tc.tile_pool(name="pv", bufs=1, space="PSUM") as pvp,
            tc.tile_pool(name="dram", bufs=2, space="DRAM") as dpool,
        ):
            # ---- warmup: ramp the PE p-state + load the exp ACT table while
            # the input DMAs stream in ----
            junk = wpool.tile([128, 640], BF16, tag="junk")
            nc.gpsimd.memset(junk[:], 1.0)
            jact = wpool.tile([128, 8], F32, tag="jact")
            nc.scalar.activation(jact[:], junk[:, :8], EXP)
            for wi in range(20):
                wps = pj.tile([128, TOK], F32, tag="pp", name="wps")
                nc.tensor.matmul(
                    wps[:, :], junk[:, :128], junk[:, 128:640], start=True, stop=True
                )

            # input streams in need-order, split across the two HW DGE queues:
            # scalar: wq, x0, c1, c3, x2; sync: c0, wk, wv, ones, c2, x1, x3
            wq_sb = wpool.tile([128, NF, DH], BF16, tag="wq")
            nc.scalar.dma_start(wq_sb[:], wqd[:])
            xt, ct = [None] * ntt, [None] * ntt
            wv_holder = {}

            def load(which, i, engine):
                if which == "wv":
                    t = wpool.tile([128, NF, DH], BF16, tag="wv")
                    engine.dma_start(t[:], wvd[:])
                    wv_holder["wv"] = t
                    return
                src = xT if which == "x" else cT
                t = spool.tile([128, NF, TOK], BF16, tag="st", name=f"{which}t{i}")
                engine.dma_start(t[:], src[:, i])
                (xt if which == "x" else ct)[i] = t

            load("x", 0, nc.scalar)
            load("c", 0, nc.sync)
            wk_sb = wpool.tile([128, NF, DH], BF16, tag="wk")
            nc.sync.dma_start(wk_sb[:], wkd[:])
            load("wv", 0, nc.sync)
            ones_sb = wpool.tile([128, HPC], BF16, tag="ones")
            nc.sync.dma_start(ones_sb[:], onesd[:])
            load("c", 1, nc.scalar)
            load("c", 2, nc.sync)
            load("c", 3, nc.scalar)
            load("x", 1, nc.sync)
            load("x", 2, nc.scalar)
            load("x", 3, nc.sync)

            # q/k as per-(o, ib) tiles for fine-grained dependencies
            qts = [[None] * ntt for _ in range(2)]
            kts = [[None] * ntt for _ in range(2)]

            def proj_chunk(w_sb, toks, o, ib, dst, tag):
                t = rpool.tile(
                    [128, TOK], BF16, tag=f"{tag}{o}_{ib}", name=f"{tag}{o}_{ib}"
                )
                pq = pj.tile([128, TOK], F32, tag="pp", name="pq")
                for f in range(NF):
                    nc.tensor.matmul(
                        pq[:, :],
                        w_sb[:, f, o * 128 : (o + 1) * 128],
                        toks[ib][:, f, :],
                        start=(f == 0),
                        stop=(f == NF - 1),
                    )
                nc.vector.tensor_copy(t[:], pq[:, :])
                dst[o][ib] = t

            def chunk_thunks(w_sb, toks, o, ib, dst, tag):
                """One projection chunk as 8 per-matmul thunks for fine-grained
                interleaving into the attention stream."""
                holder = {}
                ths = []
                for f in range(NF):

                    def th(f=f):
                        if f == 0:
                            holder["t"] = rpool.tile(
                                [128, TOK],
                                BF16,
                                tag=f"{tag}{o}_{ib}",
                                name=f"{tag}{o}_{ib}",
                            )
                            holder["pq"] = pj.tile([128, TOK], F32, tag="pp", name="pq")
                        nc.tensor.matmul(
                            holder["pq"][:, :],
                            w_sb[:, f, o * 128 : (o + 1) * 128],
                            toks[ib][:, f, :],
                            start=(f == 0),
                            stop=(f == NF - 1),
                        )
                        if f == NF - 1:
                            nc.vector.tensor_copy(holder["t"][:], holder["pq"][:, :])
                            dst[o][ib] = holder["t"]

                    ths.append(th)
                return ths

            vab = [None] * nj

            def emit_v(jc):
                # v[j, o] = sum_f cT[f,j] * WvT[f,o]; + ones column -> denominator
                pvv = pj.tile([128, TOK], F32, tag="pp", name="pvv")
                for f in range(NF):
                    nc.tensor.matmul(
                        pvv[:, :DH],
                        ct[jc // tpj][:, f, (jc % tpj) * 128 : (jc % tpj + 1) * 128],
                        wv_holder["wv"][:, f, :],
                        start=(f == 0),
                        stop=(f == NF - 1),
                    )
                va = vpool.tile([128, HPC, HD + 1], BF16, tag="vab", name="va")
                nc.vector.tensor_copy(
                    va[:, :, :HD], pvv[:, :DH].rearrange("p (h c) -> p h c", c=HD)
                )
                nc.vector.tensor_copy(va[:, :, HD : HD + 1], ones_sb[:, :, None])
                vab[jc] = va

            # minimal upfront projections: first q/k chunks + first two v chunks
            proj_chunk(wq_sb, xt, 0, 0, qts, "q")
            proj_chunk(wk_sb, ct, 0, 0, kts, "k")
            emit_v(0)
            emit_v(1)

            # remaining chunks pumped into PE slack during pair0 ih1..3
            pump_q = []
            for o, ib, wsb, tk, dst, tag in (
                (0, 2, wq_sb, xt, qts, "q"),  # needed end of ih1
                (0, 3, wq_sb, xt, qts, "q"),  # needed end of ih2
                (1, 0, wk_sb, ct, kts, "k"),  # pair1 ...
                (1, 0, wq_sb, xt, qts, "q"),
                (1, 1, wk_sb, ct, kts, "k"),
                (1, 1, wq_sb, xt, qts, "q"),
                (1, 2, wk_sb, ct, kts, "k"),
                (1, 2, wq_sb, xt, qts, "q"),
                (1, 3, wk_sb, ct, kts, "k"),
                (1, 3, wq_sb, xt, qts, "q"),
            ):
                pump_q.extend(chunk_thunks(wsb, tk, o, ib, dst, tag))
            pump_q = list(reversed(pump_q))

            def pump(n):
                for _ in range(n):
                    if pump_q:
                        pump_q.pop()()

            # ---- attention: slot per (pair, i-block, j-chunk) ----
            slots = [
                (pair, ih, jt)
                for pair in range(2)
                for ih in range(nih)
                for jt in range(nj)
            ]
            ppv_map = {}

            def emit_tail(pair, ih, last=False):
                # On the last block the exp stream is over, so the low-latency
                # HW DGE (scalar) queue is free; also split the broadcast/
                # multiply/store into head halves to pipeline DMA latencies.
                dge = nc.scalar if last else nc.gpsimd
                ppv = ppv_map.pop((pair, ih))
                psb = smpool.tile([HD + 1, 2, PW], F32, tag="psb", name="psb")
                nc.vector.tensor_copy(psb[:], ppv[:])  # frees ppv PSUM
                # reshape denominator row [1, 1024] -> [64, 16] so the DVE's
                # iterative divide (8 cyc/elem/lane) runs on 64 lanes
                rdsq = smpool.tile([64, 16], F32, tag="rdsq", name="rdsq")
                dge.dma_start(
                    rdsq[:], psb[HD : HD + 1, :, :].rearrange("p a b -> p (a b)")
                )
                rqi = smpool.tile([64, 16], F32, tag="rqi", name="rqi")
                nc.vector.reciprocal(rqi[:], rdsq[:])
                rdd = dpool.tile([1, 2 * PW], F32, tag="rdd", name="rdd")
                dge.dma_start(rdd[:], rqi[:])
                rdb = smpool.tile([64, 2, PW], F32, tag="rdb", name="rdb")
                ob = smpool.tile([64, 2, PW], F32, tag="ob", name="ob")
                for h0, h1 in ((0, 1), (1, 2)) if last else ((0, 2),):
                    rsrc = rdd[0, h0 * PW : h1 * PW]
                    bsrc = bass.AP(
                        tensor=rsrc.tensor,
                        offset=rsrc.offset,
                        ap=[[0, 64]] + list(rsrc.ap),
                    )
                    dge.dma_start(
                        rdb[:, h0:h1, :].rearrange("p a b -> p (a b)"), bsrc
                    )
                    nc.vector.tensor_mul(
                        ob[:, h0:h1, :], psb[:HD, h0:h1, :], rdb[:, h0:h1, :]
                    )
                    nc.sync.dma_start(
                        out[
                            pair * 128 + h0 * 64 : pair * 128 + h1 * 64,
                            ih * PW : (ih + 1) * PW,
                        ].rearrange("(h c) i -> c h i", h=h1 - h0),
                        ob[:, h0:h1, :],
                    )

            for pair, ih, jt in slots:
                first_block = pair == 0 and ih == 0
                if first_block:
                    if jt + 2 < nj:
                        emit_v(jt + 2)  # 2-slot lookahead over av's need
                    if jt in (1, 5, 9):
                        proj_chunk(wk_sb, ct, 0, jt // 4 + 1, kts, "k")
                    if jt == 13:
                        proj_chunk(wq_sb, xt, 0, 1, qts, "q")
                if jt == 0:
                    ppv_map[(pair, ih)] = pvp.tile(
                        [HD + 1, 2, PW], F32, tag="pv", name="ppv"
                    )
                psc = ps.tile([128, 2, PW], F32, tag="sc", name="psc")
                for hh in range(2):
                    pb = hh * 64
                    nc.tensor.matmul(
                        psc[:, hh, :],
                        kts[pair][jt // tpj][
                            pb : pb + 64, (jt % tpj) * 128 : (jt % tpj + 1) * 128
                        ],
                        qts[pair][ih][pb : pb + 64, :],
                        start=True,
                        stop=True,
                    )
                et = epool.tile([128, 2, PW], BF16, tag="et", name="et")
                nc.scalar.activation(et[:], psc[:], EXP)
                if not first_block:
                    pump(2)
                ppv = ppv_map[(pair, ih)]
                for hh in range(2):
                    nc.tensor.matmul(
                        ppv[:, hh, :],
                        vab[jt][:, pair * 2 + hh, :],
                        et[:, hh, :],
                        start=(jt == 0),
                        stop=(jt == nj - 1),
                    )
                if jt == nj - 1:
                    emit_tail(pair, ih, last=(pair == 1 and ih == nih - 1))
            pump(len(pump_q))  # safety: flush anything left

    if split_waits:
        _split_excess_waits(nc)
    return nc


def make_in_maps(x, context, Wq, Wkv, s=S):
    """Host-side shard + layout prep. Core c -> (batch c//HG, head group c%HG).

    x/context ship as [128, ntt, NF, TOK] and weights as [128, NF, DH] so all
    DMA reads are contiguous per partition."""
    x = np.asarray(x, dtype=np.float32)
    context = np.asarray(context, dtype=np.float32)
    Wq = np.asarray(Wq, dtype=np.float32)
    Wkv = np.asarray(Wkv, dtype=np.float32)
    scale = np.float32(HD**-0.5)
    ntt = s // TOK

    def arrange_tok(a):  # [s, D] -> [128, ntt, NF, TOK]
        return np.ascontiguousarray(
            a.T.reshape(NF, 128, ntt, TOK).transpose(1, 2, 0, 3)
        ).astype(BF)

    def arrange_w(w):  # [DH, D] -> [128, NF, DH]
        return np.ascontiguousarray(w.T.reshape(NF, 128, DH).transpose(1, 0, 2)).astype(
            BF
        )

    in_maps = []
    for core in range(N_CORES):
        b, hg = core // HG, core % HG
        sl = slice(hg * DH, (hg + 1) * DH)
        in_maps.append(
            {
                "xT": arrange_tok(x[b]),
                "cT": arrange_tok(context[b]),
                "wqd": arrange_w(Wq[sl] * scale),
                "wkd": arrange_w(Wkv[sl]),
                "wvd": arrange_w(Wkv[D + hg * DH : D + (hg + 1) * DH]),
                "onesd": np.ones((128, HPC), dtype=BF),
            }
        )
    return in_maps


def gather_out(results, s=S):
    full = np.empty((B, s, D), dtype=np.float32)
    for core in range(N_CORES):
        b, hg = core // HG, core % HG
        full[b, :, hg * DH : (hg + 1) * DH] = results[core]["out"].T
    return full


def kernel(x, context, Wq, Wkv):
    nc = build_nc(S)
    in_maps = make_in_maps(x, context, Wq, Wkv, S)
    res = run_bass_kernel_spmd(nc, in_maps, list(range(N_CORES)))
    return gather_out(res.results, S)
